# revision 5
# baseline (speedup 1.0000x reference)
"""Antisymmetric RNN kernel for Trainium2, data-parallel over batch on 8 cores.

Math (reference):
    M = W - W^T - gamma*I
    h_t = x_t @ V + bias                      [B, U]
    state_{t+1} = state_t + eps*tanh(h_t + state_t @ M)
    out[:, t] = state_{t+1}

Device formulation (per core, B_local=16):
    Rescale S' = state/eps, M' = eps*M  =>  S'_{t+1} = S'_t + tanh(z_t),
    z_t = h_t + S'_t @ M'. Keep everything transposed: partitions carry u
    (2 chunks of 128), free dim carries (chunk, batch) = 32 columns.

    A single PSUM tile Z [128, 32] holds z_t across the whole run,
    updated in place by PE matmuls only:
      init: Z = bias + (x0/eps) @ M' + x_0 @ V        (k=2 bias matmul + 6 mms)
      step: th_t = tanh(Z)                  (ScalarE, PSUM -> SBUF bf16)
            Z += M'[k,c] @ th_t[k]  (4 mms) (advances S' contribution)
            Z += V@x_{t+1} - V@x_t  (4 mms) (x-window swap; exact cancellation)
    The output states S'_{t+1} = x0/eps + cumsum_t(th_t) are reconstructed
    off the critical path with DVE tensor_tensor_scan over the tanh history,
    then DMA'd out. Host multiplies by eps and re-layouts.
"""

import sys

sys.path.insert(0, "/opt/trn_rl_repo")

import numpy as np
import ml_dtypes

import concourse.bass as bass
import concourse.bacc as bacc
import concourse.mybir as mybir
import concourse.tile as tile

EPS = 0.01
GAMMA = 0.01
B, T, D, U = 128, 1024, 128, 256
NCORES = 8
BL = B // NCORES  # 16 batch rows per core
NK = U // 128  # 2 u-chunks
W32 = NK * BL  # 32 free columns = (chunk, batch)
CH = 256  # history chunk (timesteps) per scan/DMA-out block

F32 = mybir.dt.float32
BF16 = mybir.dt.bfloat16
BF16_NP = ml_dtypes.bfloat16

_CACHED = {}


def build_nc(t_steps=T):
    nc = bacc.Bacc(None, target_bir_lowering=False)
    x_d = nc.declare_dram_parameter("xT", [D, t_steps, BL], BF16, isOutput=False)
    m_d = nc.declare_dram_parameter("Mp", [128, NK, NK, 128], BF16, isOutput=False)
    v_d = nc.declare_dram_parameter("Vp", [D, 2, NK, 128], BF16, isOutput=False)
    b_d = nc.declare_dram_parameter("b2", [NK, 128], BF16, isOutput=False)
    s_d = nc.declare_dram_parameter("sel", [NK, W32], BF16, isOutput=False)
    xt_d = nc.declare_dram_parameter("x0t", [128, NK, BL], BF16, isOutput=False)
    xh_d = nc.declare_dram_parameter("x0h", [128, W32], F32, isOutput=False)
    o_d = nc.declare_dram_parameter("out", [128, t_steps, W32], F32, isOutput=True)

    Tanh = mybir.ActivationFunctionType.Tanh
    ADD = mybir.AluOpType.add
    BYPASS = mybir.AluOpType.bypass

    with tile.TileContext(nc) as tc:
        with (
            tc.tile_pool(name="const", bufs=1) as cpool,
            tc.tile_pool(name="xp", bufs=1) as xpool,
            tc.tile_pool(name="tb", bufs=1) as tbpool,
            tc.tile_pool(name="hist", bufs=2) as hpool,
            tc.tile_pool(name="ps", bufs=1, space=bass.MemorySpace.PSUM) as ppool,
        ):
            m_sb = cpool.tile([128, NK, NK, 128], BF16)
            v_sb = cpool.tile([D, 2, NK, 128], BF16)
            b_sb = cpool.tile([NK, 128], BF16)
            s_sb = cpool.tile([NK, W32], BF16)
            xt_sb = cpool.tile([128, NK, BL], BF16)
            xh_sb = cpool.tile([128, W32], F32)
            nc.sync.dma_start(m_sb[:], m_d[:])
            nc.sync.dma_start(v_sb[:], v_d[:])
            nc.sync.dma_start(b_sb[:], b_d[:])
            nc.sync.dma_start(s_sb[:], s_d[:])
            nc.sync.dma_start(xt_sb[:], xt_d[:])
            nc.sync.dma_start(xh_sb[:], xh_d[:])

            x_sb = xpool.tile([D, t_steps, BL], BF16)
            xch = 128 if t_steps % 128 == 0 else t_steps
            for i in range(t_steps // xch):
                sl = slice(i * xch, (i + 1) * xch)
                nc.sync.dma_start(x_sb[:, sl, :], x_d[:, sl, :])

            tb_sb = tbpool.tile([128, t_steps, W32], BF16)
            # Two mirrored PSUM accumulators (separate banks). ACT reads one
            # bank while PE applies catch-up updates to the other, so only
            # the 4 tanh-dependent M-matmuls sit on the serial chain.
            z_bank_a = ppool.tile([128, W32], F32, tag="zA")
            z_bank_b = ppool.tile([128, W32], F32, tag="zB")
            z_banks = [z_bank_a, z_bank_b]

            def emit_xswap(zb, s):
                # h window swap: += V @ x_{s+1} - V @ x_s  (exactly telescopes)
                for c in range(NK):
                    zc = zb[:, c * BL : (c + 1) * BL]
                    nc.tensor.matmul(
                        zc, v_sb[:, 0, c, :], x_sb[:, s + 1, :], start=False, stop=False
                    )
                    nc.tensor.matmul(
                        zc, v_sb[:, 1, c, :], x_sb[:, s, :], start=False, stop=False
                    )

            def emit_m(zb, s, stop=False):
                # += M'[k,c] @ tanh_s[k]
                for c in range(NK):
                    zc = zb[:, c * BL : (c + 1) * BL]
                    for k in range(NK):
                        last = stop and c == NK - 1 and k == NK - 1
                        nc.tensor.matmul(
                            zc,
                            m_sb[:, k, c, :],
                            tb_sb[:, s, k * BL : (k + 1) * BL],
                            start=False,
                            stop=last,
                        )

            z_ps = z_banks[0]

            # ---- init both banks: Z_0 = bias + (x0/eps) @ M' + x_0 @ V ----
            for zb in z_banks:
                nc.tensor.matmul(zb[:], b_sb[:], s_sb[:], start=True, stop=False)
                for c in range(NK):
                    zc = zb[:, c * BL : (c + 1) * BL]
                    for k in range(NK):
                        nc.tensor.matmul(
                            zc, m_sb[:, k, c, :], xt_sb[:, k, :], start=False,
                            stop=False,
                        )
                    nc.tensor.matmul(
                        zc, v_sb[:, 0, c, :], x_sb[:, 0, :], start=False, stop=False
                    )

            # ---- recurrence ----
            # step t: ACT reads bank t%2; the other bank is advanced from its
            # Z_{t-1} state to Z_{t+1}: catch-up upd_{t-1} and x-swap_t are
            # ready when the tanh starts (they overlap it); only M-mms_t are
            # on the serial chain.
            prev_hist = None
            for t in range(t_steps):
                p = z_banks[t % 2]
                q = z_banks[(t + 1) % 2]
                nc.scalar.activation(tb_sb[:, t, :], p[:], Tanh)
                if t < t_steps - 1:
                    if t >= 1:
                        emit_xswap(q, t - 1)
                        emit_m(q, t - 1, stop=False)
                    emit_xswap(q, t)
                    emit_m(q, t, stop=(t >= t_steps - 3))
                # ---- chunk epilogue: scan tanh history into states, DMA out ----
                if (t + 1) % CH == 0 or t == t_steps - 1:
                    ch_len = CH if (t + 1) % CH == 0 else (t + 1) % CH
                    c0 = t + 1 - ch_len
                    hist = hpool.tile([128, CH, W32], F32, tag="hist")
                    for j in range(W32):
                        if prev_hist is None:
                            init = xh_sb[:, j : j + 1]
                        else:
                            init = prev_hist[:, CH - 1, j : j + 1]
                        nc.vector.tensor_tensor_scan(
                            hist[:, :ch_len, j],
                            tb_sb[:, c0 : t + 1, j],
                            tb_sb[:, c0 : t + 1, j],
                            init,
                            ADD,
                            BYPASS,
                        )
                    nc.sync.dma_start(
                        o_d[:, c0 : t + 1, :], hist[:, :ch_len, :]
                    )
                    prev_hist = hist

    nc.compile()
    return nc


def _prep_consts(V, W, bias, x0):
    M = W - W.T - GAMMA * np.eye(U, dtype=np.float32)
    Mp = (EPS * M).reshape(NK, 128, NK, 128).transpose(1, 0, 2, 3)
    Vr = V.reshape(D, NK, 128)
    Vp = np.stack([Vr, -Vr], axis=1)  # [D, 2, NK, 128]
    b2 = bias.reshape(NK, 128)
    sel = np.zeros((NK, W32), dtype=np.float32)
    for c in range(NK):
        sel[c, c * BL : (c + 1) * BL] = 1.0
    x0e = (x0 / EPS).astype(np.float32)
    x0t = np.broadcast_to(x0e.reshape(NK, 128).transpose(1, 0)[:, :, None], (128, NK, BL))
    x0h = np.ascontiguousarray(x0t).reshape(128, W32)
    return {
        "Mp": np.ascontiguousarray(Mp).astype(BF16_NP),
        "Vp": np.ascontiguousarray(Vp).astype(BF16_NP),
        "b2": np.ascontiguousarray(b2).astype(BF16_NP),
        "sel": np.ascontiguousarray(sel).astype(BF16_NP),
        "x0t": np.ascontiguousarray(x0t).astype(BF16_NP),
        "x0h": np.ascontiguousarray(x0h).astype(np.float32),
    }


def _install_ntff_hook():
    # Register the axon NTFF profile hook if the image's antenv lacks it,
    # so trace=True can return exec_time_ns. Harmless if anything fails.
    import types

    try:
        import antenv.axon_hooks  # noqa: F401

        return
    except ImportError:
        pass
    try:
        import antenv
        from trn_agent_boot.trn_boot import _ntff_profile_via_ctypes

        mod = types.ModuleType("antenv.axon_hooks")
        _h = [None]
        mod.set_axon_ntff_profile_hook = lambda h: _h.__setitem__(0, h)
        mod.get_axon_ntff_profile_hook = lambda: _h[0]
        sys.modules["antenv.axon_hooks"] = mod
        antenv.axon_hooks = mod
        mod.set_axon_ntff_profile_hook(
            _ntff_profile_via_ctypes("/opt/axon/libaxon_pjrt.so")
        )
    except Exception:
        pass


def kernel(inputs, V, W, bias, x0, _t_steps=None, _trace=False):
    _install_ntff_hook()
    from concourse.bass_utils import run_bass_kernel_spmd

    inputs = np.asarray(inputs, dtype=np.float32)
    V = np.asarray(V, dtype=np.float32)
    W = np.asarray(W, dtype=np.float32)
    bias = np.asarray(bias, dtype=np.float32)
    x0 = np.asarray(x0, dtype=np.float32)

    t_steps = _t_steps or inputs.shape[1]
    key = t_steps
    if key not in _CACHED:
        _CACHED[key] = build_nc(t_steps)
    nc = _CACHED[key]

    consts = _prep_consts(V, W, bias, x0)
    in_maps = []
    for i in range(NCORES):
        shard = inputs[i * BL : (i + 1) * BL, :t_steps, :]  # [16, t, 128]
        xT = np.ascontiguousarray(shard.transpose(2, 1, 0)).astype(BF16_NP)
        in_maps.append({"xT": xT, **consts})

    res = run_bass_kernel_spmd(
        nc, in_maps, list(range(NCORES)), trace=_trace
    )
    outs = []
    for i in range(NCORES):
        o = res.results[i]["out"]  # [128, t, 32] f32
        o = o.reshape(128, t_steps, NK, BL).transpose(3, 1, 2, 0).reshape(BL, t_steps, U)
        outs.append(o)
    full = np.concatenate(outs, axis=0) * EPS
    if _trace:
        return full.astype(np.float32), res
    return full.astype(np.float32)
